# revision 17
# baseline (speedup 1.0000x reference)
"""DGL-style 2-layer GAT on 8 TRN2 NeuronCores (Bass/Tile).

Design (v2): dst nodes + incident edges partitioned across 8 cores.

L1 (no collectives, no indirect DMA): the host stages, per core, the
transposed x rows for every edge slot (XE1, [256, T1*128] bf16, edge
slots grouped into 49 windows of 128 dsts, LPT-balanced).  Per
128-edge tile the PE computes F = x_e @ [W1 | W1·al-blk] -> PSUM
[128,260]; a dst-indicator matmul M2^T @ er_w accumulates er into the
el columns, giving z = el[src]+er[dst] in PSUM; DVE applies leaky-relu,
ACT exponentiates into ees, DVE scales feat by ee per head, and a
second indicator matmul M1^T @ ees accumulates [ee*feat | ee] per dst
into PSUM.  er_w is precomputed on-device from own-dst x (XD1 @ W1·ar).

L2: feat2 = h @ [W2 | W2·al | W2·ar] computed per window right after
the L1 flush (h transposed via PE), AllGathered (the only collective),
then per-tile indirect row gathers feed the same indicator-matmul edge
pipeline; output = mean over heads of u/s.
"""
import sys
sys.path.insert(0, '/opt/trn_rl_repo')

import numpy as np
import ml_dtypes

import concourse.bass as bass
import concourse.tile as tile
from concourse import bacc, mybir
from concourse.masks import make_identity

P = 128
NCORES = 8
N0, N1, N2 = 100000, 50000, 8000
E0, E1 = 600000, 80000
F_IN, HID, H, C = 256, 64, 4, 47
NEG = 0.2

BLK1 = N1 // NCORES        # 6250 dst1 per core
BLK2 = N2 // NCORES        # 1000 dst2 per core
NW1 = 49                   # L1 windows (49*128 = 6272 slots)
NW2 = 8                    # L2 windows (8*128 = 1024 slots)
SL1 = NW1 * P              # 6272
SL2 = NW2 * P              # 1024
NF1 = 256                  # L1 feat cols
NF2 = 188                  # L2 feat cols
GROW2 = 196                # G2 row: 188 feat + 4 el + 4 er (bf16)
PADRD = 200.0              # rd for pad edge slots (no dst match)
NCH = 7                    # AllGather chunks (7 windows each)
CH = NW1 // NCH            # windows per chunk

F32 = mybir.dt.float32
BF16 = mybir.dt.bfloat16
I32 = mybir.dt.int32
AF = mybir.ActivationFunctionType
OP = mybir.AluOpType
BF = ml_dtypes.bfloat16

_cache = {}


def build_program(tw1, tw2, add_b1, add_b2, chdep2):
    """tw1/tw2: per-window tile counts; chdep2[t]: AG chunk each L2 tile
    needs (max over cores)."""
    key = (tuple(tw1), tuple(tw2), add_b1, add_b2, tuple(chdep2))
    if key in _cache:
        return _cache[key]
    T1, T2 = sum(tw1), sum(tw2)
    nc = bacc.Bacc("TRN2", num_devices=NCORES)
    # ---- I/O
    XE1 = nc.declare_dram_parameter("XE1", [F_IN, T1 * P], BF16, isOutput=False)
    XD1 = nc.declare_dram_parameter("XD1", [F_IN, SL1], BF16, isOutput=False)
    RD1 = nc.declare_dram_parameter("RD1", [1, T1 * P], BF16, isOutput=False)
    RD2 = nc.declare_dram_parameter("RD2", [1, T2 * P], BF16, isOutput=False)
    MRC1 = nc.declare_dram_parameter("MRC1", [P, T1], BF16, isOutput=False)
    MRC2 = nc.declare_dram_parameter("MRC2", [P, T2], BF16, isOutput=False)
    GIDX2 = nc.declare_dram_parameter("GIDX2", [P, T2], I32, isOutput=False)
    E2I = nc.declare_dram_parameter("E2I", [P, NW2], I32, isOutput=False)
    W1E = nc.declare_dram_parameter("W1E", [F_IN, NF1 + 4], BF16, isOutput=False)
    WAR = nc.declare_dram_parameter("WAR", [F_IN, 4], BF16, isOutput=False)
    W2E = nc.declare_dram_parameter("W2E", [F_IN, GROW2], BF16, isOutput=False)
    B1R = nc.declare_dram_parameter("B1R", [P, NF1], F32, isOutput=False)
    B2R = nc.declare_dram_parameter("B2R", [P, C], F32, isOutput=False)
    OUT = nc.declare_dram_parameter("OUT", [SL2, C], F32, isOutput=True)
    # ---- internal DRAM
    G2in = nc.dram_tensor("G2in", [SL1, GROW2], BF16)
    G2C = [nc.dram_tensor(f"G2C{c}", [NCORES * CH * P, GROW2], BF16,
                          addr_space="Shared") for c in range(NCH)]

    with tile.TileContext(nc) as tc:
        with (
            tc.tile_pool(name="const", bufs=1) as const,
            tc.tile_pool(name="sb", bufs=3) as sb,
        ):
            iota_i = const.tile([P, P], I32)
            nc.gpsimd.iota(iota_i[:], pattern=[[1, P]], base=0, channel_multiplier=0)
            iota_f = const.tile([P, P], BF16)
            nc.vector.tensor_copy(out=iota_f[:], in_=iota_i[:])
            iota_pi = const.tile([P, P], I32)
            nc.gpsimd.iota(iota_pi[:], pattern=[[0, P]], base=0, channel_multiplier=1)
            iota_p = const.tile([P, P], BF16)
            nc.vector.tensor_copy(out=iota_p[:], in_=iota_pi[:])
            iota_f2 = const.tile([P, 2 * P], BF16)
            nc.vector.tensor_copy(out=iota_f2[:, 0:P], in_=iota_f[:])
            nc.vector.tensor_copy(out=iota_f2[:, P:2 * P], in_=iota_f[:])
            iota_p2 = const.tile([P, 2 * P], BF16)
            nc.vector.tensor_copy(out=iota_p2[:, 0:P], in_=iota_p[:])
            nc.vector.tensor_copy(out=iota_p2[:, P:2 * P], in_=iota_p[:])
            ident = const.tile([P, P], BF16)
            make_identity(nc, ident[:])
            w1e = [const.tile([P, NF1 + 4], BF16, name=f"w1e{k}", tag=f"w1e{k}")
                   for k in range(2)]
            war = [const.tile([P, 4], BF16, name=f"war{k}", tag=f"war{k}")
                   for k in range(2)]
            w2e = [const.tile([P, GROW2], BF16, name=f"w2e{k}", tag=f"w2e{k}")
                   for k in range(2)]
            for k in range(2):
                nc.sync.dma_start(out=w1e[k][:], in_=W1E[k * P:(k + 1) * P, :])
                nc.sync.dma_start(out=war[k][:], in_=WAR[k * P:(k + 1) * P, :])
                nc.sync.dma_start(out=w2e[k][:], in_=W2E[k * P:(k + 1) * P, :])
            mrc1 = const.tile([P, T1], BF16)
            nc.sync.dma_start(out=mrc1[:], in_=MRC1[:])
            mrc2 = const.tile([P, T2], BF16)
            nc.sync.dma_start(out=mrc2[:], in_=MRC2[:])
            gidx2 = const.tile([P, T2], I32)
            nc.sync.dma_start(out=gidx2[:], in_=GIDX2[:])
            e2i = const.tile([P, NW2], I32)
            nc.sync.dma_start(out=e2i[:], in_=E2I[:])
            if add_b1:
                b1t = const.tile([P, NF1], F32)
                nc.sync.dma_start(out=b1t[:], in_=B1R[:])
            if add_b2:
                b2t = const.tile([P, C], F32)
                nc.sync.dma_start(out=b2t[:], in_=B2R[:])
            ers1 = const.tile([P, NW1 * 4], BF16)
            ers2 = const.tile([P, NW2 * 4], BF16)
            hT = [const.tile([P, SL1], BF16, name=f"hT{k}", tag=f"hT{k}")
                  for k in range(2)]

            # ============ phase A: er1 table (own-dst x @ W1*ar) ============
            with (tc.tile_pool(name="xdp", bufs=1) as xdp,
                  tc.tile_pool(name="psA", bufs=2, space="PSUM") as ps):
                xd = [xdp.tile([P, SL1], BF16, name=f"xd{k}", tag=f"xd{k}")
                      for k in range(2)]
                for k in range(2):
                    nc.sync.dma_start(out=xd[k][:], in_=XD1[k * P:(k + 1) * P, :])
                for w in range(NW1):
                    erp = ps.tile([P, 4], F32, tag="erp")
                    for k in range(2):
                        nc.tensor.matmul(out=erp[:],
                                         lhsT=xd[k][:, w * P:(w + 1) * P],
                                         rhs=war[k][:],
                                         start=(k == 0), stop=(k == 1))
                    nc.vector.tensor_copy(out=ers1[:, w * 4:(w + 1) * 4], in_=erp[:])

            # ============ phase B: L1 edge windows (+ inline phase4) ========
            g2_writes = []
            cc_list = []
            LA = 3
            with (
                tc.tile_pool(name="xep", bufs=3) as xep,
                tc.tile_pool(name="rdp", bufs=2) as rdp,
                tc.tile_pool(name="mp", bufs=4) as mp,
                tc.tile_pool(name="zp", bufs=5) as zp,
                tc.tile_pool(name="eep", bufs=5) as eep,
                tc.tile_pool(name="psF", bufs=4, space="PSUM") as psF,
                tc.tile_pool(name="psAcc", bufs=2, space="PSUM") as psAcc,
                tc.tile_pool(name="psT", bufs=1, space="PSUM") as psT,
                tc.tile_pool(name="psP", bufs=1, space="PSUM") as psP,
            ):
                off = 0
                for w in range(NW1):
                    tw = tw1[w]
                    cols = slice(off * P, (off + tw) * P)
                    xe = [xep.tile([P, tw * P], BF16, name=f"xe{k}", tag=f"xe{k}")
                          for k in range(2)]
                    nc.sync.dma_start(out=xe[0][:], in_=XE1[0:P, cols])
                    nc.sync.dma_start(out=xe[1][:], in_=XE1[P:2 * P, cols])
                    rdb = rdp.tile([P, tw * P], BF16, tag="rdb")
                    nc.sync.dma_start(out=rdb[:],
                                       in_=RD1[0:1, cols].to_broadcast([P, tw * P]))
                    acc = psAcc.tile([P, NF1 + 4], F32, tag="acc")
                    Mp = {}
                    ees_q = {}

                    def stageA(j):
                        t = off + j
                        jj = j % 2
                        if jj == 0:
                            nb = min(2, tw - j)
                            M2p = mp.tile([P, 2 * P], BF16, tag="m2p")
                            nc.vector.tensor_tensor(
                                out=M2p[:, 0:nb * P], in0=iota_p2[:, 0:nb * P],
                                in1=rdb[:, j * P:(j + nb) * P], op=OP.is_equal)
                            M1p = mp.tile([P, 2 * P], BF16, tag="m1p")
                            nc.vector.tensor_tensor(
                                out=M1p[:, 0:nb * P].rearrange(
                                    "p (b q) -> p b q", b=nb),
                                in0=iota_f2[:, 0:nb * P].rearrange(
                                    "p (b q) -> p b q", b=nb),
                                in1=mrc1[:, t:t + nb][:, :, None].broadcast_to(
                                    [P, nb, P]),
                                op=OP.is_equal)
                            Mp[j] = (M1p, M2p)
                        M1p, M2p = Mp[j - jj]
                        F = psF.tile([P, NF1 + 4], F32, tag="F")
                        for k in range(2):
                            nc.tensor.matmul(out=F[:],
                                             lhsT=xe[k][:, j * P:(j + 1) * P],
                                             rhs=w1e[k][:],
                                             start=(k == 0), stop=False)
                        nc.tensor.matmul(out=F[:, NF1:NF1 + 4],
                                         lhsT=M2p[:, jj * P:(jj + 1) * P],
                                         rhs=ers1[:, w * 4:(w + 1) * 4],
                                         start=False, stop=True)
                        ee1 = zp.tile([P, 4], F32, tag="ee1")
                        nc.scalar.activation(out=ee1[:], in_=F[:, NF1:NF1 + 4],
                                             func=AF.Exp)
                        ee2 = zp.tile([P, 4], F32, tag="ee2")
                        nc.scalar.activation(out=ee2[:], in_=F[:, NF1:NF1 + 4],
                                             func=AF.Exp, scale=NEG)
                        ees = eep.tile([P, NF1 + 4], BF16, tag="ees")
                        if j % 5 == 4:
                            # offload ee*feat to ACT via per-head scale copies
                            eef = zp.tile([P, 4], F32, tag="eef")
                            nc.vector.tensor_tensor(out=eef[:], in0=ee1[:],
                                                    in1=ee2[:], op=OP.max)
                            nc.vector.tensor_copy(out=ees[:, NF1:NF1 + 4],
                                                  in_=eef[:])
                            for hh in range(H):
                                nc.scalar.activation(
                                    out=ees[:, hh * HID:(hh + 1) * HID],
                                    in_=F[:, hh * HID:(hh + 1) * HID],
                                    func=AF.Copy, scale=eef[:, hh:hh + 1])
                        else:
                            nc.vector.tensor_tensor(out=ees[:, NF1:NF1 + 4],
                                                    in0=ee1[:], in1=ee2[:],
                                                    op=OP.max)
                            nc.vector.tensor_tensor(
                                out=ees[:, 0:NF1].rearrange("p (h d) -> p h d", h=H),
                                in0=F[:, 0:NF1].rearrange("p (h d) -> p h d", h=H),
                                in1=ees[:, NF1:NF1 + 4][:, :, None].broadcast_to(
                                    [P, H, HID]),
                                op=OP.mult)
                        ees_q[j] = (M1p, jj, ees)

                    for j in range(min(LA, tw)):
                        stageA(j)
                    for j in range(tw):
                        if j + LA < tw:
                            stageA(j + LA)
                        M1p, jj, ees = ees_q.pop(j)
                        nc.tensor.matmul(out=acc[:],
                                         lhsT=M1p[:, jj * P:(jj + 1) * P],
                                         rhs=ees[:], start=(j == 0),
                                         stop=(j == tw - 1))
                    off += tw
                    # -------- flush1: h = elu(u/s), transpose into hT --------
                    sden = sb.tile([P, 4], F32, tag="sden")
                    nc.vector.tensor_scalar_max(out=sden[:], in0=acc[:, NF1:NF1 + 4],
                                                scalar1=1e-30)
                    nc.vector.reciprocal(out=sden[:], in_=sden[:])
                    z = sb.tile([P, NF1], BF16, tag="z")
                    nc.vector.tensor_tensor(
                        out=z[:].rearrange("p (h d) -> p h d", h=H),
                        in0=acc[:, 0:NF1].rearrange("p (h d) -> p h d", h=H),
                        in1=sden[:, :, None].broadcast_to([P, H, HID]), op=OP.mult)
                    if add_b1:
                        nc.vector.tensor_tensor(out=z[:], in0=z[:], in1=b1t[:],
                                                op=OP.add)
                    zm2 = sb.tile([P, NF1], BF16, tag="zm2")
                    nc.vector.tensor_scalar_min(out=zm2[:], in0=z[:], scalar1=0.0)
                    nc.scalar.activation(out=zm2[:], in_=zm2[:], func=AF.Exp)
                    hb = sb.tile([P, NF1], BF16, tag="hb")
                    nc.vector.tensor_scalar(out=hb[:], in0=z[:], scalar1=0.0,
                                            scalar2=-1.0, op0=OP.max, op1=OP.add)
                    nc.gpsimd.tensor_tensor(out=hb[:], in0=hb[:], in1=zm2[:],
                                            op=OP.add)
                    for k in range(2):
                        tp = psT.tile([P, P], BF16, tag="tp")
                        nc.tensor.transpose(out=tp[:], in_=hb[:, k * P:(k + 1) * P],
                                            identity=ident[:])
                        nc.scalar.activation(out=hT[k][:, w * P:(w + 1) * P],
                                             in_=tp[:], func=AF.Copy)
                    # -------- phase4 (inline): feat2 for this window ---------
                    pm2 = psP.tile([P, GROW2], F32, tag="pm2")
                    for k in range(2):
                        nc.tensor.matmul(out=pm2[:],
                                         lhsT=hT[k][:, w * P:(w + 1) * P],
                                         rhs=w2e[k][:],
                                         start=(k == 0), stop=(k == 1))
                    gs2 = sb.tile([P, GROW2], BF16, tag="gs2")
                    nc.scalar.activation(out=gs2[:], in_=pm2[:], func=AF.Copy)
                    d1 = nc.scalar.dma_start(out=G2in[w * P:(w + 1) * P, :],
                                             in_=gs2[:])
                    g2_writes.append(d1)
                    if (w + 1) % CH == 0:
                        c = w // CH
                        rows = slice(c * CH * P, (c + 1) * CH * P)
                        cc = nc.gpsimd.collective_compute(
                            "AllGather", OP.bypass,
                            replica_groups=[list(range(NCORES))],
                            ins=[G2in[rows]], outs=[G2C[c][:]])
                        for d in g2_writes[c * CH:(c + 1) * CH]:
                            tile.add_dep_helper(cc.ins, d.ins, sync=True)
                        cc_list.append(cc)

            # ======= phase C: chunked AllGather G2 (overlaps L1 tail) =======

            # ============ phase E: L2 edge windows ============
            with (
                tc.tile_pool(name="gp", bufs=1) as gp,
                tc.tile_pool(name="rdp2", bufs=1) as rdp2,
                tc.tile_pool(name="mp2", bufs=4) as mp2,
                tc.tile_pool(name="zp2", bufs=5) as zp2,
                tc.tile_pool(name="eep2", bufs=5) as eep2,
                tc.tile_pool(name="psE", bufs=2, space="PSUM") as ps,
            ):
                rdb2a = rdp2.tile([P, T2 * P], BF16, tag="rdb2")
                nc.sync.dma_start(out=rdb2a[:],
                                  in_=RD2[0:1, :].to_broadcast([P, T2 * P]))
                # prefetch ALL edge-row gathers in chunk order (avoids
                # head-of-line blocking on the in-order Pool queue); slot
                # the local er2 gathers at their readiness point.
                gba = gp.tile([P, T2, GROW2], BF16, tag="gba")

                def emit_gather(t):
                    i1 = nc.gpsimd.indirect_dma_start(
                        out=gba[:, t, :], out_offset=None,
                        in_=G2C[chdep2[t]][:],
                        in_offset=bass.IndirectOffsetOnAxis(
                            ap=gidx2[:, t:t + 1], axis=0))
                    tile.add_dep_helper(i1.ins, cc_list[chdep2[t]].ins,
                                        sync=True)

                gorder = sorted(range(T2), key=lambda t: chdep2[t])
                for t in gorder:
                    if chdep2[t] <= NCH - 3:
                        emit_gather(t)
                with tc.tile_pool(name="e2p", bufs=2) as e2p:
                    for w in range(NW2):
                        g2c = e2p.tile([P, GROW2], BF16, tag="g2c")
                        i1 = nc.gpsimd.indirect_dma_start(
                            out=g2c[:], out_offset=None, in_=G2in[:],
                            in_offset=bass.IndirectOffsetOnAxis(
                                ap=e2i[:, w:w + 1], axis=0))
                        tile.add_dep_helper(i1.ins, g2_writes[-1].ins, sync=True)
                        nc.vector.tensor_copy(out=ers2[:, w * 4:(w + 1) * 4],
                                              in_=g2c[:, NF2 + 4:NF2 + 8])
                for t in gorder:
                    if chdep2[t] > NCH - 3:
                        emit_gather(t)

                off = 0
                for w in range(NW2):
                    tw = tw2[w]
                    rdb = rdb2a[:, off * P:(off + tw) * P]
                    acc = ps.tile([P, NF2 + 4], F32, tag="acc2")
                    Mp = {}
                    ees_q = {}

                    def stageA2(j):
                        t = off + j
                        jj = j % 2
                        gb = gba[:, t, :]
                        if jj == 0:
                            nb = min(2, tw - j)
                            M2p = mp2.tile([P, 2 * P], BF16, tag="m22p")
                            nc.vector.tensor_tensor(
                                out=M2p[:, 0:nb * P], in0=iota_p2[:, 0:nb * P],
                                in1=rdb[:, j * P:(j + nb) * P], op=OP.is_equal)
                            M1p = mp2.tile([P, 2 * P], BF16, tag="m12p")
                            nc.vector.tensor_tensor(
                                out=M1p[:, 0:nb * P].rearrange(
                                    "p (b q) -> p b q", b=nb),
                                in0=iota_f2[:, 0:nb * P].rearrange(
                                    "p (b q) -> p b q", b=nb),
                                in1=mrc2[:, t:t + nb][:, :, None].broadcast_to(
                                    [P, nb, P]),
                                op=OP.is_equal)
                            Mp[j] = (M1p, M2p)
                        M1p, M2p = Mp[j - jj]
                        er2p = ps.tile([P, 4], F32, tag="er2p")
                        nc.tensor.matmul(out=er2p[:], lhsT=M2p[:, jj * P:(jj + 1) * P],
                                         rhs=ers2[:, w * 4:(w + 1) * 4],
                                         start=True, stop=True)
                        zs = zp2.tile([P, 4], F32, tag="zs2")
                        nc.vector.tensor_tensor(out=zs[:], in0=er2p[:],
                                                in1=gb[:, NF2:NF2 + 4], op=OP.add)
                        ee1 = zp2.tile([P, 4], F32, tag="e21")
                        nc.scalar.activation(out=ee1[:], in_=zs[:], func=AF.Exp)
                        ee2 = zp2.tile([P, 4], F32, tag="e22")
                        nc.scalar.activation(out=ee2[:], in_=zs[:], func=AF.Exp,
                                             scale=NEG)
                        ees = eep2.tile([P, NF2 + 4], BF16, tag="ees2")
                        nc.vector.tensor_tensor(out=ees[:, NF2:NF2 + 4], in0=ee1[:],
                                                in1=ee2[:], op=OP.max)
                        nc.gpsimd.tensor_tensor(
                            out=ees[:, 0:NF2].rearrange("p (h c) -> p h c", h=H),
                            in0=gb[:, 0:NF2].rearrange("p (h c) -> p h c", h=H),
                            in1=ees[:, NF2:NF2 + 4][:, :, None].broadcast_to(
                                [P, H, C]),
                            op=OP.mult)
                        ees_q[j] = (M1p, jj, ees)

                    for j in range(min(LA, tw)):
                        stageA2(j)
                    for j in range(tw):
                        if j + LA < tw:
                            stageA2(j + LA)
                        M1p, jj, ees = ees_q.pop(j)
                        nc.tensor.matmul(out=acc[:],
                                         lhsT=M1p[:, jj * P:(jj + 1) * P],
                                         rhs=ees[:], start=(j == 0),
                                         stop=(j == tw - 1))
                    off += tw
                    # -------- flush2: out = mean_h(u/s) --------
                    sden = sb.tile([P, 4], F32, tag="sden2")
                    nc.vector.tensor_scalar_max(out=sden[:], in0=acc[:, NF2:NF2 + 4],
                                                scalar1=1e-30)
                    nc.vector.reciprocal(out=sden[:], in_=sden[:])
                    nc.vector.tensor_scalar_mul(out=sden[:], in0=sden[:],
                                                scalar1=1.0 / H)
                    z2 = sb.tile([P, NF2], F32, tag="z2")
                    nc.vector.tensor_tensor(
                        out=z2[:].rearrange("p (h c) -> p h c", h=H),
                        in0=acc[:, 0:NF2].rearrange("p (h c) -> p h c", h=H),
                        in1=sden[:, :, None].broadcast_to([P, H, C]), op=OP.mult)
                    o = sb.tile([P, C], F32, tag="o")
                    nc.vector.tensor_reduce(
                        out=o[:], in_=z2[:].rearrange("p (h c) -> p c h", h=H),
                        axis=mybir.AxisListType.X, op=OP.add)
                    if add_b2:
                        nc.vector.tensor_tensor(out=o[:], in0=o[:], in1=b2t[:],
                                                op=OP.add)
                    nc.sync.dma_start(out=OUT[w * P:(w + 1) * P, :], in_=o[:])

    nc.compile()
    _cache[key] = nc
    return nc


def _blkdiag(a):
    o = np.zeros((a.shape[0] * a.shape[1], a.shape[0]), np.float32)
    for h in range(a.shape[0]):
        o[h * a.shape[1]:(h + 1) * a.shape[1], h] = a[h]
    return o


def _core_lpt(deg, count_cap, pinned=None):
    """Assign nodes (degree desc) to NCORES cores, balancing total degree,
    count per core <= count_cap.  pinned[n] >= 0 forces node n's core.
    Returns core_of [n]."""
    import heapq
    core_of = np.zeros(len(deg), np.int64)
    loads = np.zeros(NCORES, np.int64)
    counts = np.zeros(NCORES, np.int64)
    free = np.ones(len(deg), bool)
    if pinned is not None:
        sel = pinned >= 0
        core_of[sel] = pinned[sel]
        free[sel] = False
        for r in range(NCORES):
            loads[r] = int(deg[(pinned == r)].sum())
            counts[r] = int((pinned == r).sum())
    order = np.argsort(-deg, kind='stable')
    heap = [(int(loads[r]), int(counts[r]), r) for r in range(NCORES)]
    heapq.heapify(heap)
    for d in order:
        if not free[d]:
            continue
        while True:
            load, cnt, r = heapq.heappop(heap)
            if cnt != counts[r]:
                continue  # stale entry
            if counts[r] < count_cap:
                break
        core_of[d] = r
        counts[r] += 1
        loads[r] += int(deg[d])
        heapq.heappush(heap, (int(loads[r]), int(counts[r]), r))
    return core_of


def _lpt_pack(dst_local, n_local, nwin, caps=None):
    """Pack dsts (by degree desc) into nwin windows (<=128 dsts each),
    respecting per-window edge-load caps when feasible.  Returns
    slot_of_dst [n_local] (w*128+p) and per-window edge loads."""
    deg = np.bincount(dst_local, minlength=n_local)
    order = np.argsort(-deg, kind='stable')
    loads = np.zeros(nwin, np.int64)
    counts = np.zeros(nwin, np.int64)
    slot = np.zeros(n_local, np.int64)
    import heapq
    heap = [(0, 0, w) for w in range(nwin)]
    heapq.heapify(heap)
    for d in order:
        dg = int(deg[d])
        popped = []
        chosen = None
        while heap:
            load, cnt, w = heapq.heappop(heap)
            if counts[w] >= P:
                continue  # full window: drop from heap for good
            if caps is not None and loads[w] + dg > caps[w]:
                popped.append((load, cnt, w))
                continue
            chosen = w
            break
        if chosen is None:
            # no window satisfies the cap: use min-load non-full window
            popped.sort()
            chosen = popped.pop(0)[2]
        for item in popped:
            heapq.heappush(heap, item)
        w = chosen
        slot[d] = w * P + counts[w]
        counts[w] += 1
        loads[w] += dg
        heapq.heappush(heap, (int(loads[w]), int(counts[w]), w))
    return slot, loads


def kernel(x, W1, al1, ar1, b1, W2, al2, ar2, b2, src0, dst0, src1, dst1):
    x = np.asarray(x, np.float32); W1 = np.asarray(W1, np.float32)
    al1 = np.asarray(al1, np.float32); ar1 = np.asarray(ar1, np.float32)
    b1 = np.asarray(b1, np.float32); W2 = np.asarray(W2, np.float32)
    al2 = np.asarray(al2, np.float32); ar2 = np.asarray(ar2, np.float32)
    b2 = np.asarray(b2, np.float32)
    src0 = np.asarray(src0, np.int32); dst0 = np.asarray(dst0, np.int32)
    src1 = np.asarray(src1, np.int32); dst1 = np.asarray(dst1, np.int32)

    W1Ev = np.concatenate([W1, W1 @ _blkdiag(al1)], axis=1).astype(BF)
    WARv = (W1 @ _blkdiag(ar1)).astype(BF)
    W2Ev = np.concatenate([W2, W2 @ _blkdiag(al2), W2 @ _blkdiag(ar2)],
                          axis=1).astype(BF)
    b1r = np.broadcast_to(b1.reshape(1, NF1), (P, NF1)).astype(np.float32).copy()
    b2m = b2.reshape(H, C).mean(axis=0)
    b2r = np.broadcast_to(b2m.reshape(1, C), (P, C)).astype(np.float32).copy()
    add_b1 = bool(np.any(b1)); add_b2 = bool(np.any(b2))
    xbf = x.astype(BF)

    # ---- node->core assignment: LPT by degree (count cap = slot count).
    # dst2 nodes are pinned to the same core for L1 and L2 so er2 comes
    # from the local G2in slab (no AllGather dependency).
    core2_of = _core_lpt(np.bincount(dst1, minlength=N2), SL2)
    deg1 = np.bincount(dst0, minlength=N1)
    pinned = np.full(N1, -1, np.int64)
    pinned[:N2] = core2_of
    core1_of = _core_lpt(deg1, SL1, pinned=pinned)
    core1 = core1_of[dst0]
    core2 = core2_of[dst1]

    # ---- per-core L1 window packing
    slot1 = np.zeros(N1, np.int64)       # dst1 -> w*128+p (within owning core)
    packs1 = []
    nodes1 = [np.flatnonzero(core1_of == r) for r in range(NCORES)]
    for r in range(NCORES):
        sel = core1 == r
        loc = np.full(N1, -1, np.int64)
        loc[nodes1[r]] = np.arange(len(nodes1[r]))
        dl = loc[dst0[sel]]
        s, loads = _lpt_pack(dl, len(nodes1[r]), NW1)
        slot1[nodes1[r]] = s
        packs1.append((np.flatnonzero(sel), dl, s, loads))
    # canonical window order: sort windows by load desc per core
    worder1 = [np.argsort(-p[3], kind='stable') for p in packs1]
    lw1 = np.stack([p[3][wo] for p, wo in zip(packs1, worder1)])  # [cores, NW1]
    tw1 = [max(1, int(np.ceil(lw1[:, w].max() / P))) for w in range(NW1)]
    # remap slot window ids to canonical order
    for r in range(NCORES):
        inv = np.empty(NW1, np.int64)
        inv[worder1[r]] = np.arange(NW1)
        s = slot1[nodes1[r]]
        slot1[nodes1[r]] = inv[s // P] * P + s % P

    # dst1 node -> G2 row under chunked-AllGather layout:
    # row = chunk*(8*CH*128) + core*(CH*128) + (w % CH)*128 + p
    _w = slot1 // P
    _p = slot1 % P
    _core = core1_of
    g2row = ((_w // CH) * (NCORES * CH * P) + _core * (CH * P)
             + (_w % CH) * P + _p)

    # ---- per-core L2 window packing
    slot2 = np.zeros(N2, np.int64)
    packs2 = []
    nodes2 = [np.flatnonzero(core2_of == r) for r in range(NCORES)]
    for r in range(NCORES):
        sel = core2 == r
        loc = np.full(N2, -1, np.int64)
        loc[nodes2[r]] = np.arange(len(nodes2[r]))
        dl = loc[dst1[sel]]
        s, loads = _lpt_pack(dl, len(nodes2[r]), NW2)
        slot2[nodes2[r]] = s
        packs2.append((np.flatnonzero(sel), dl, s, loads))
    worder2 = [np.argsort(-p[3], kind='stable') for p in packs2]
    lw2 = np.stack([p[3][wo] for p, wo in zip(packs2, worder2)])
    tw2 = [max(1, int(np.ceil(lw2[:, w].max() / P))) for w in range(NW2)]
    for r in range(NCORES):
        inv = np.empty(NW2, np.int64)
        inv[worder2[r]] = np.arange(NW2)
        s = slot2[nodes2[r]]
        slot2[nodes2[r]] = inv[s // P] * P + s % P

    # ---- chunk-pure L2 tiling: per (window, chunk) tile runs, profile
    # shared across cores (max) so the SPMD program lines up.
    CHROWS = NCORES * CH * P
    cnt2 = np.zeros((NCORES, NW2, NCH), np.int64)
    edata = []
    for r in range(NCORES):
        eidx2 = packs2[r][0]
        sl = slot2[dst1[eidx2]]
        wv = sl // P
        rows = g2row[src1[eidx2]]
        cv = rows // CHROWS
        np.add.at(cnt2[r], (wv, cv), 1)
        edata.append((sl, rows, cv, wv))
    ntile2 = np.ceil(cnt2.max(axis=0) / P).astype(np.int64)  # [NW2, NCH]
    for w in range(NW2):
        if ntile2[w].sum() == 0:
            ntile2[w, 0] = 1
    tw2 = [int(ntile2[w].sum()) for w in range(NW2)]
    chdep2 = []
    for w in range(NW2):
        for c in range(NCH):
            chdep2.extend([c] * int(ntile2[w, c]))
    T1, T2 = sum(tw1), sum(tw2)
    woff1 = np.concatenate([[0], np.cumsum(tw1)])
    woff2 = np.concatenate([[0], np.cumsum(tw2)])
    roff2 = np.zeros((NW2, NCH), np.int64)   # tile offset of run (w, c)
    for w in range(NW2):
        roff2[w] = woff2[w] + np.concatenate([[0], np.cumsum(ntile2[w])[:-1]])

    def pack_l2(r):
        sl, rows, cv, wv = edata[r]
        src_sl = np.zeros(T2 * P, np.int64)
        rd = np.full(T2 * P, PADRD, np.float32)
        order = np.lexsort((cv, wv))
        key = wv[order] * NCH + cv[order]
        bounds = np.searchsorted(key, np.arange(NW2 * NCH + 1))
        for w in range(NW2):
            for c in range(NCH):
                a, b = bounds[w * NCH + c], bounds[w * NCH + c + 1]
                n = b - a
                if n == 0:
                    continue
                idx = order[a:b]
                base = roff2[w, c] * P
                assert n <= ntile2[w, c] * P
                src_sl[base:base + n] = rows[idx] % CHROWS
                rd[base:base + n] = sl[idx] % P
        return src_sl, rd

    def pack_edges(eidx, srcrows, slots, nwin, tws, woff, T, sortkey=None):
        """Lay edges into window slot arrays.  Returns src_slot [T*P],
        rd [T*P] f32, tile_maxkey [T] (max sortkey per tile, 0 for empty)."""
        src_sl = np.zeros(T * P, np.int64)
        rd = np.full(T * P, PADRD, np.float32)
        tile_maxkey = np.zeros(T, np.int64)
        w_of_e = slots[:] // P
        if sortkey is None:
            order = np.argsort(w_of_e, kind='stable')
        else:
            order = np.lexsort((sortkey, w_of_e))
        bounds = np.searchsorted(w_of_e[order], np.arange(nwin + 1))
        for w in range(nwin):
            a, b = bounds[w], bounds[w + 1]
            idx = order[a:b]
            n = b - a
            base = woff[w] * P
            assert n <= tws[w] * P, (w, n, tws[w] * P)
            src_sl[base:base + n] = srcrows[idx]
            rd[base:base + n] = slots[idx] % P
            if sortkey is not None and n > 0:
                keys = sortkey[idx]
                for j in range((n + P - 1) // P):
                    tile_maxkey[woff[w] + j] = keys[min((j + 1) * P, n) - 1]
        return src_sl, rd, tile_maxkey

    in_maps = []
    for r in range(NCORES):
        eidx1, dl1, _, _ = packs1[r]
        sl_e1 = slot1[dst0[eidx1]]          # slot of each edge's dst
        src_sl1, rd1, _ = pack_edges(eidx1, src0[eidx1], sl_e1, NW1, tw1,
                                     woff1, T1)
        XE1v = xbf[src_sl1].T.copy()        # [256, T1*128]
        own1 = np.zeros(SL1, np.int64)      # window slot -> x row (own dsts)
        dsts_r = nodes1[r]
        own1[slot1[dsts_r]] = dsts_r
        XD1v = xbf[own1].T.copy()
        RD1v = rd1.astype(BF).reshape(1, -1).copy()
        MRC1v = rd1.astype(BF).reshape(T1, P).T.copy()

        eidx2 = packs2[r][0]
        src_sl2, rd2 = pack_l2(r)
        GIDX2v = src_sl2.astype(np.int32).reshape(T2, P).T.copy()
        RD2v = rd2.astype(BF).reshape(1, -1).copy()
        MRC2v = rd2.astype(BF).reshape(T2, P).T.copy()
        own2 = np.zeros(SL2, np.int64)
        dsts2_r = nodes2[r]
        own2[slot2[dsts2_r]] = slot1[dsts2_r]   # local G2in row
        E2Iv = own2.astype(np.int32).reshape(NW2, P).T.copy()

        in_maps.append(dict(
            XE1=XE1v, XD1=XD1v, RD1=RD1v, RD2=RD2v, MRC1=MRC1v, MRC2=MRC2v,
            GIDX2=GIDX2v, E2I=E2Iv, W1E=W1Ev, WAR=WARv, W2E=W2Ev,
            B1R=b1r, B2R=b2r))

    global _last_in_maps
    _last_in_maps = in_maps
    nc = build_program(tw1, tw2, add_b1, add_b2, chdep2)
    from concourse.bass_utils import run_bass_kernel_spmd
    res = None
    last_err = None
    for attempt in range(3):
        try:
            res = run_bass_kernel_spmd(nc, in_maps, core_ids=list(range(NCORES)))
            break
        except Exception as e:
            last_err = e
            import time as _t
            _t.sleep(10)
    if res is None:
        raise last_err
    out = np.zeros((N2, C), np.float32)
    for r in range(NCORES):
        o = res.results[r]["OUT"]
        dsts2_r = nodes2[r]
        out[dsts2_r] = o[slot2[dsts2_r]]
    return out


# revision 20
# speedup vs baseline: 9.1224x; 9.1224x over previous
"""DGL-style 2-layer GAT on 8 TRN2 NeuronCores (Bass/Tile).

Design (v2): dst nodes + incident edges partitioned across 8 cores.

L1 (no collectives, no indirect DMA): the host stages, per core, the
transposed x rows for every edge slot (XE1, [256, T1*128] bf16, edge
slots grouped into 49 windows of 128 dsts, LPT-balanced).  Per
128-edge tile the PE computes F = x_e @ [W1 | W1·al-blk] -> PSUM
[128,260]; a dst-indicator matmul M2^T @ er_w accumulates er into the
el columns, giving z = el[src]+er[dst] in PSUM; DVE applies leaky-relu,
ACT exponentiates into ees, DVE scales feat by ee per head, and a
second indicator matmul M1^T @ ees accumulates [ee*feat | ee] per dst
into PSUM.  er_w is precomputed on-device from own-dst x (XD1 @ W1·ar).

L2: feat2 = h @ [W2 | W2·al | W2·ar] computed per window right after
the L1 flush (h transposed via PE), AllGathered (the only collective),
then per-tile indirect row gathers feed the same indicator-matmul edge
pipeline; output = mean over heads of u/s.
"""
import sys
sys.path.insert(0, '/opt/trn_rl_repo')

import numpy as np
import ml_dtypes

import concourse.bass as bass
import concourse.tile as tile
from concourse import bacc, mybir
from concourse.masks import make_identity

P = 128
NCORES = 8
N0, N1, N2 = 100000, 50000, 8000
E0, E1 = 600000, 80000
F_IN, HID, H, C = 256, 64, 4, 47
NEG = 0.2

BLK1 = N1 // NCORES        # 6250 dst1 per core
BLK2 = N2 // NCORES        # 1000 dst2 per core
NW1 = 49                   # L1 windows (49*128 = 6272 slots)
NW2 = 8                    # L2 windows (8*128 = 1024 slots)
SL1 = NW1 * P              # 6272
SL2 = NW2 * P              # 1024
NF1 = 256                  # L1 feat cols
NF2 = 188                  # L2 feat cols
GROW2 = 196                # G2 row: 188 feat + 4 el + 4 er (bf16)
PADRD = 200.0              # rd for pad edge slots (no dst match)
NCH = 7                    # AllGather chunks (7 windows each)
CH = NW1 // NCH            # windows per chunk

F32 = mybir.dt.float32
BF16 = mybir.dt.bfloat16
I32 = mybir.dt.int32
AF = mybir.ActivationFunctionType
OP = mybir.AluOpType
BF = ml_dtypes.bfloat16

_cache = {}


def build_program(tw1, tw2, add_b1, add_b2, chdep2):
    """tw1/tw2: per-window tile counts; chdep2[t]: AG chunk each L2 tile
    needs (max over cores)."""
    key = (tuple(tw1), tuple(tw2), add_b1, add_b2, tuple(chdep2))
    if key in _cache:
        return _cache[key]
    T1, T2 = sum(tw1), sum(tw2)
    nc = bacc.Bacc("TRN2", num_devices=NCORES)
    # ---- I/O
    XE1 = nc.declare_dram_parameter("XE1", [F_IN, T1 * P], BF16, isOutput=False)
    XD1 = nc.declare_dram_parameter("XD1", [F_IN, SL1], BF16, isOutput=False)
    RD1 = nc.declare_dram_parameter("RD1", [1, T1 * P], BF16, isOutput=False)
    RD2 = nc.declare_dram_parameter("RD2", [1, T2 * P], BF16, isOutput=False)
    MRC1 = nc.declare_dram_parameter("MRC1", [P, T1], BF16, isOutput=False)
    MRC2 = nc.declare_dram_parameter("MRC2", [P, T2], BF16, isOutput=False)
    GIDX2 = nc.declare_dram_parameter("GIDX2", [P, T2], I32, isOutput=False)
    E2I = nc.declare_dram_parameter("E2I", [P, NW2], I32, isOutput=False)
    W1E = nc.declare_dram_parameter("W1E", [F_IN, NF1 + 4], BF16, isOutput=False)
    WAR = nc.declare_dram_parameter("WAR", [F_IN, 4], BF16, isOutput=False)
    W2E = nc.declare_dram_parameter("W2E", [F_IN, GROW2], BF16, isOutput=False)
    B1R = nc.declare_dram_parameter("B1R", [P, NF1], F32, isOutput=False)
    B2R = nc.declare_dram_parameter("B2R", [P, C], F32, isOutput=False)
    OUT = nc.declare_dram_parameter("OUT", [SL2, C], F32, isOutput=True)
    # ---- internal DRAM
    G2in = nc.dram_tensor("G2in", [SL1, GROW2], BF16)
    G2C = [nc.dram_tensor(f"G2C{c}", [NCORES * CH * P, GROW2], BF16,
                          addr_space="Shared") for c in range(NCH)]

    with tile.TileContext(nc) as tc:
        with (
            tc.tile_pool(name="const", bufs=1) as const,
            tc.tile_pool(name="sb", bufs=3) as sb,
        ):
            iota_i = const.tile([P, P], I32)
            nc.gpsimd.iota(iota_i[:], pattern=[[1, P]], base=0, channel_multiplier=0)
            iota_f = const.tile([P, P], BF16)
            nc.vector.tensor_copy(out=iota_f[:], in_=iota_i[:])
            iota_pi = const.tile([P, P], I32)
            nc.gpsimd.iota(iota_pi[:], pattern=[[0, P]], base=0, channel_multiplier=1)
            iota_p = const.tile([P, P], BF16)
            nc.vector.tensor_copy(out=iota_p[:], in_=iota_pi[:])
            iota_f2 = const.tile([P, 2 * P], BF16)
            nc.vector.tensor_copy(out=iota_f2[:, 0:P], in_=iota_f[:])
            nc.vector.tensor_copy(out=iota_f2[:, P:2 * P], in_=iota_f[:])
            iota_p2 = const.tile([P, 2 * P], BF16)
            nc.vector.tensor_copy(out=iota_p2[:, 0:P], in_=iota_p[:])
            nc.vector.tensor_copy(out=iota_p2[:, P:2 * P], in_=iota_p[:])
            ident = const.tile([P, P], BF16)
            make_identity(nc, ident[:])
            w1e = [const.tile([P, NF1 + 4], BF16, name=f"w1e{k}", tag=f"w1e{k}")
                   for k in range(2)]
            war = [const.tile([P, 4], BF16, name=f"war{k}", tag=f"war{k}")
                   for k in range(2)]
            w2e = [const.tile([P, GROW2], BF16, name=f"w2e{k}", tag=f"w2e{k}")
                   for k in range(2)]
            for k in range(2):
                nc.sync.dma_start(out=w1e[k][:], in_=W1E[k * P:(k + 1) * P, :])
                nc.sync.dma_start(out=war[k][:], in_=WAR[k * P:(k + 1) * P, :])
                nc.sync.dma_start(out=w2e[k][:], in_=W2E[k * P:(k + 1) * P, :])
            mrc1 = const.tile([P, T1], BF16)
            nc.sync.dma_start(out=mrc1[:], in_=MRC1[:])
            mrc2 = const.tile([P, T2], BF16)
            nc.sync.dma_start(out=mrc2[:], in_=MRC2[:])
            gidx2 = const.tile([P, T2], I32)
            nc.sync.dma_start(out=gidx2[:], in_=GIDX2[:])
            e2i = const.tile([P, NW2], I32)
            nc.sync.dma_start(out=e2i[:], in_=E2I[:])
            if add_b1:
                b1t = const.tile([P, NF1], F32)
                nc.sync.dma_start(out=b1t[:], in_=B1R[:])
            if add_b2:
                b2t = const.tile([P, C], F32)
                nc.sync.dma_start(out=b2t[:], in_=B2R[:])
            ers1 = const.tile([P, NW1 * 4], BF16)
            ers2 = const.tile([P, NW2 * 4], BF16)
            hT = [const.tile([P, SL1], BF16, name=f"hT{k}", tag=f"hT{k}")
                  for k in range(2)]

            # ============ phase A: er1 table (own-dst x @ W1*ar) ============
            with (tc.tile_pool(name="xdp", bufs=1) as xdp,
                  tc.tile_pool(name="psA", bufs=2, space="PSUM") as ps):
                xd = [xdp.tile([P, SL1], BF16, name=f"xd{k}", tag=f"xd{k}")
                      for k in range(2)]
                for k in range(2):
                    nc.sync.dma_start(out=xd[k][:], in_=XD1[k * P:(k + 1) * P, :])
                for w in range(NW1):
                    erp = ps.tile([P, 4], F32, tag="erp")
                    for k in range(2):
                        nc.tensor.matmul(out=erp[:],
                                         lhsT=xd[k][:, w * P:(w + 1) * P],
                                         rhs=war[k][:],
                                         start=(k == 0), stop=(k == 1))
                    nc.vector.tensor_copy(out=ers1[:, w * 4:(w + 1) * 4], in_=erp[:])

            # ============ phase B: L1 edge windows (+ inline phase4) ========
            g2_writes = []
            cc_list = []
            LA = 3
            with (
                tc.tile_pool(name="xep", bufs=3) as xep,
                tc.tile_pool(name="rdp", bufs=2) as rdp,
                tc.tile_pool(name="mp", bufs=4) as mp,
                tc.tile_pool(name="zp", bufs=5) as zp,
                tc.tile_pool(name="eep", bufs=5) as eep,
                tc.tile_pool(name="psF", bufs=4, space="PSUM") as psF,
                tc.tile_pool(name="psAcc", bufs=2, space="PSUM") as psAcc,
                tc.tile_pool(name="psT", bufs=1, space="PSUM") as psT,
                tc.tile_pool(name="psP", bufs=1, space="PSUM") as psP,
            ):
                off = 0
                for w in range(NW1):
                    tw = tw1[w]
                    cols = slice(off * P, (off + tw) * P)
                    xe = [xep.tile([P, tw * P], BF16, name=f"xe{k}", tag=f"xe{k}")
                          for k in range(2)]
                    nc.sync.dma_start(out=xe[0][:], in_=XE1[0:P, cols])
                    nc.sync.dma_start(out=xe[1][:], in_=XE1[P:2 * P, cols])
                    rdb = rdp.tile([P, tw * P], BF16, tag="rdb")
                    nc.sync.dma_start(out=rdb[:],
                                       in_=RD1[0:1, cols].to_broadcast([P, tw * P]))
                    acc = psAcc.tile([P, NF1 + 4], F32, tag="acc")
                    Mp = {}
                    ees_q = {}

                    def stageA(j):
                        t = off + j
                        jj = j % 2
                        if jj == 0:
                            nb = min(2, tw - j)
                            M2p = mp.tile([P, 2 * P], BF16, tag="m2p")
                            nc.vector.tensor_tensor(
                                out=M2p[:, 0:nb * P], in0=iota_p2[:, 0:nb * P],
                                in1=rdb[:, j * P:(j + nb) * P], op=OP.is_equal)
                            M1p = mp.tile([P, 2 * P], BF16, tag="m1p")
                            nc.vector.tensor_tensor(
                                out=M1p[:, 0:nb * P].rearrange(
                                    "p (b q) -> p b q", b=nb),
                                in0=iota_f2[:, 0:nb * P].rearrange(
                                    "p (b q) -> p b q", b=nb),
                                in1=mrc1[:, t:t + nb][:, :, None].broadcast_to(
                                    [P, nb, P]),
                                op=OP.is_equal)
                            Mp[j] = (M1p, M2p)
                        M1p, M2p = Mp[j - jj]
                        F = psF.tile([P, NF1 + 4], F32, tag="F")
                        for k in range(2):
                            nc.tensor.matmul(out=F[:],
                                             lhsT=xe[k][:, j * P:(j + 1) * P],
                                             rhs=w1e[k][:],
                                             start=(k == 0), stop=False)
                        nc.tensor.matmul(out=F[:, NF1:NF1 + 4],
                                         lhsT=M2p[:, jj * P:(jj + 1) * P],
                                         rhs=ers1[:, w * 4:(w + 1) * 4],
                                         start=False, stop=True)
                        ee1 = zp.tile([P, 4], F32, tag="ee1")
                        nc.scalar.activation(out=ee1[:], in_=F[:, NF1:NF1 + 4],
                                             func=AF.Exp)
                        ee2 = zp.tile([P, 4], F32, tag="ee2")
                        nc.scalar.activation(out=ee2[:], in_=F[:, NF1:NF1 + 4],
                                             func=AF.Exp, scale=NEG)
                        ees = eep.tile([P, NF1 + 4], BF16, tag="ees")
                        if j % 5 == 4:
                            # offload ee*feat to ACT via per-head scale copies
                            eef = zp.tile([P, 4], F32, tag="eef")
                            nc.vector.tensor_tensor(out=eef[:], in0=ee1[:],
                                                    in1=ee2[:], op=OP.max)
                            nc.vector.tensor_copy(out=ees[:, NF1:NF1 + 4],
                                                  in_=eef[:])
                            for hh in range(H):
                                nc.scalar.activation(
                                    out=ees[:, hh * HID:(hh + 1) * HID],
                                    in_=F[:, hh * HID:(hh + 1) * HID],
                                    func=AF.Copy, scale=eef[:, hh:hh + 1])
                        else:
                            nc.vector.tensor_tensor(out=ees[:, NF1:NF1 + 4],
                                                    in0=ee1[:], in1=ee2[:],
                                                    op=OP.max)
                            nc.vector.tensor_tensor(
                                out=ees[:, 0:NF1].rearrange("p (h d) -> p h d", h=H),
                                in0=F[:, 0:NF1].rearrange("p (h d) -> p h d", h=H),
                                in1=ees[:, NF1:NF1 + 4][:, :, None].broadcast_to(
                                    [P, H, HID]),
                                op=OP.mult)
                        ees_q[j] = (M1p, jj, ees)

                    for j in range(min(LA, tw)):
                        stageA(j)
                    for j in range(tw):
                        if j + LA < tw:
                            stageA(j + LA)
                        M1p, jj, ees = ees_q.pop(j)
                        nc.tensor.matmul(out=acc[:],
                                         lhsT=M1p[:, jj * P:(jj + 1) * P],
                                         rhs=ees[:], start=(j == 0),
                                         stop=(j == tw - 1))
                    off += tw
                    # -------- flush1: h = elu(u/s), transpose into hT --------
                    sden = sb.tile([P, 4], F32, tag="sden")
                    nc.vector.tensor_scalar_max(out=sden[:], in0=acc[:, NF1:NF1 + 4],
                                                scalar1=1e-30)
                    nc.vector.reciprocal(out=sden[:], in_=sden[:])
                    z = sb.tile([P, NF1], BF16, tag="z")
                    nc.vector.tensor_tensor(
                        out=z[:].rearrange("p (h d) -> p h d", h=H),
                        in0=acc[:, 0:NF1].rearrange("p (h d) -> p h d", h=H),
                        in1=sden[:, :, None].broadcast_to([P, H, HID]), op=OP.mult)
                    if add_b1:
                        nc.vector.tensor_tensor(out=z[:], in0=z[:], in1=b1t[:],
                                                op=OP.add)
                    zm2 = sb.tile([P, NF1], BF16, tag="zm2")
                    nc.vector.tensor_scalar_min(out=zm2[:], in0=z[:], scalar1=0.0)
                    nc.scalar.activation(out=zm2[:], in_=zm2[:], func=AF.Exp)
                    hb = sb.tile([P, NF1], BF16, tag="hb")
                    nc.vector.tensor_scalar(out=hb[:], in0=z[:], scalar1=0.0,
                                            scalar2=-1.0, op0=OP.max, op1=OP.add)
                    nc.gpsimd.tensor_tensor(out=hb[:], in0=hb[:], in1=zm2[:],
                                            op=OP.add)
                    for k in range(2):
                        tp = psT.tile([P, P], BF16, tag="tp")
                        nc.tensor.transpose(out=tp[:], in_=hb[:, k * P:(k + 1) * P],
                                            identity=ident[:])
                        nc.scalar.activation(out=hT[k][:, w * P:(w + 1) * P],
                                             in_=tp[:], func=AF.Copy)
                    # -------- phase4 (inline): feat2 for this window ---------
                    pm2 = psP.tile([P, GROW2], F32, tag="pm2")
                    for k in range(2):
                        nc.tensor.matmul(out=pm2[:],
                                         lhsT=hT[k][:, w * P:(w + 1) * P],
                                         rhs=w2e[k][:],
                                         start=(k == 0), stop=(k == 1))
                    gs2 = sb.tile([P, GROW2], BF16, tag="gs2")
                    nc.scalar.activation(out=gs2[:], in_=pm2[:], func=AF.Copy)
                    d1 = nc.scalar.dma_start(out=G2in[w * P:(w + 1) * P, :],
                                             in_=gs2[:])
                    g2_writes.append(d1)
                    if (w + 1) % CH == 0:
                        c = w // CH
                        rows = slice(c * CH * P, (c + 1) * CH * P)
                        cc = nc.gpsimd.collective_compute(
                            "AllGather", OP.bypass,
                            replica_groups=[list(range(NCORES))],
                            ins=[G2in[rows]], outs=[G2C[c][:]])
                        for d in g2_writes[c * CH:(c + 1) * CH]:
                            tile.add_dep_helper(cc.ins, d.ins, sync=True)
                        cc_list.append(cc)

            # ======= phase C: chunked AllGather G2 (overlaps L1 tail) =======

            # ============ phase E: L2 edge windows ============
            with (
                tc.tile_pool(name="gp", bufs=1) as gp,
                tc.tile_pool(name="rdp2", bufs=1) as rdp2,
                tc.tile_pool(name="mp2", bufs=4) as mp2,
                tc.tile_pool(name="zp2", bufs=5) as zp2,
                tc.tile_pool(name="eep2", bufs=5) as eep2,
                tc.tile_pool(name="psE", bufs=2, space="PSUM") as ps,
            ):
                rdb2a = rdp2.tile([P, T2 * P], BF16, tag="rdb2")
                nc.sync.dma_start(out=rdb2a[:],
                                  in_=RD2[0:1, :].to_broadcast([P, T2 * P]))
                # prefetch ALL edge-row gathers in chunk order (avoids
                # head-of-line blocking on the in-order Pool queue); slot
                # the local er2 gathers at their readiness point.
                gba = gp.tile([P, T2, GROW2], BF16, tag="gba")

                def emit_gather(t):
                    i1 = nc.gpsimd.indirect_dma_start(
                        out=gba[:, t, :], out_offset=None,
                        in_=G2C[chdep2[t]][:],
                        in_offset=bass.IndirectOffsetOnAxis(
                            ap=gidx2[:, t:t + 1], axis=0))
                    tile.add_dep_helper(i1.ins, cc_list[chdep2[t]].ins,
                                        sync=True)

                gorder = sorted(range(T2), key=lambda t: chdep2[t])
                for t in gorder:
                    if chdep2[t] <= NCH - 3:
                        emit_gather(t)
                with tc.tile_pool(name="e2p", bufs=2) as e2p:
                    for w in range(NW2):
                        g2c = e2p.tile([P, GROW2], BF16, tag="g2c")
                        i1 = nc.gpsimd.indirect_dma_start(
                            out=g2c[:], out_offset=None, in_=G2in[:],
                            in_offset=bass.IndirectOffsetOnAxis(
                                ap=e2i[:, w:w + 1], axis=0))
                        tile.add_dep_helper(i1.ins, g2_writes[-1].ins, sync=True)
                        nc.vector.tensor_copy(out=ers2[:, w * 4:(w + 1) * 4],
                                              in_=g2c[:, NF2 + 4:NF2 + 8])
                for t in gorder:
                    if chdep2[t] > NCH - 3:
                        emit_gather(t)

                off = 0
                for w in range(NW2):
                    tw = tw2[w]
                    rdb = rdb2a[:, off * P:(off + tw) * P]
                    acc = ps.tile([P, NF2 + 4], F32, tag="acc2")
                    Mp = {}
                    ees_q = {}

                    def stageA2(j):
                        t = off + j
                        jj = j % 2
                        gb = gba[:, t, :]
                        if jj == 0:
                            nb = min(2, tw - j)
                            M2p = mp2.tile([P, 2 * P], BF16, tag="m22p")
                            nc.vector.tensor_tensor(
                                out=M2p[:, 0:nb * P], in0=iota_p2[:, 0:nb * P],
                                in1=rdb[:, j * P:(j + nb) * P], op=OP.is_equal)
                            M1p = mp2.tile([P, 2 * P], BF16, tag="m12p")
                            nc.vector.tensor_tensor(
                                out=M1p[:, 0:nb * P].rearrange(
                                    "p (b q) -> p b q", b=nb),
                                in0=iota_f2[:, 0:nb * P].rearrange(
                                    "p (b q) -> p b q", b=nb),
                                in1=mrc2[:, t:t + nb][:, :, None].broadcast_to(
                                    [P, nb, P]),
                                op=OP.is_equal)
                            Mp[j] = (M1p, M2p)
                        M1p, M2p = Mp[j - jj]
                        er2p = ps.tile([P, 4], F32, tag="er2p")
                        nc.tensor.matmul(out=er2p[:], lhsT=M2p[:, jj * P:(jj + 1) * P],
                                         rhs=ers2[:, w * 4:(w + 1) * 4],
                                         start=True, stop=True)
                        zs = zp2.tile([P, 4], F32, tag="zs2")
                        nc.vector.tensor_tensor(out=zs[:], in0=er2p[:],
                                                in1=gb[:, NF2:NF2 + 4], op=OP.add)
                        ee1 = zp2.tile([P, 4], F32, tag="e21")
                        nc.scalar.activation(out=ee1[:], in_=zs[:], func=AF.Exp)
                        ee2 = zp2.tile([P, 4], F32, tag="e22")
                        nc.scalar.activation(out=ee2[:], in_=zs[:], func=AF.Exp,
                                             scale=NEG)
                        ees = eep2.tile([P, NF2 + 4], BF16, tag="ees2")
                        nc.vector.tensor_tensor(out=ees[:, NF2:NF2 + 4], in0=ee1[:],
                                                in1=ee2[:], op=OP.max)
                        nc.gpsimd.tensor_tensor(
                            out=ees[:, 0:NF2].rearrange("p (h c) -> p h c", h=H),
                            in0=gb[:, 0:NF2].rearrange("p (h c) -> p h c", h=H),
                            in1=ees[:, NF2:NF2 + 4][:, :, None].broadcast_to(
                                [P, H, C]),
                            op=OP.mult)
                        ees_q[j] = (M1p, jj, ees)

                    for j in range(min(LA, tw)):
                        stageA2(j)
                    for j in range(tw):
                        if j + LA < tw:
                            stageA2(j + LA)
                        M1p, jj, ees = ees_q.pop(j)
                        nc.tensor.matmul(out=acc[:],
                                         lhsT=M1p[:, jj * P:(jj + 1) * P],
                                         rhs=ees[:], start=(j == 0),
                                         stop=(j == tw - 1))
                    off += tw
                    # -------- flush2: out = mean_h(u/s) --------
                    sden = sb.tile([P, 4], F32, tag="sden2")
                    nc.vector.tensor_scalar_max(out=sden[:], in0=acc[:, NF2:NF2 + 4],
                                                scalar1=1e-30)
                    nc.vector.reciprocal(out=sden[:], in_=sden[:])
                    nc.vector.tensor_scalar_mul(out=sden[:], in0=sden[:],
                                                scalar1=1.0 / H)
                    z2 = sb.tile([P, NF2], F32, tag="z2")
                    nc.vector.tensor_tensor(
                        out=z2[:].rearrange("p (h c) -> p h c", h=H),
                        in0=acc[:, 0:NF2].rearrange("p (h c) -> p h c", h=H),
                        in1=sden[:, :, None].broadcast_to([P, H, C]), op=OP.mult)
                    o = sb.tile([P, C], F32, tag="o")
                    nc.vector.tensor_reduce(
                        out=o[:], in_=z2[:].rearrange("p (h c) -> p c h", h=H),
                        axis=mybir.AxisListType.X, op=OP.add)
                    if add_b2:
                        nc.vector.tensor_tensor(out=o[:], in0=o[:], in1=b2t[:],
                                                op=OP.add)
                    nc.sync.dma_start(out=OUT[w * P:(w + 1) * P, :], in_=o[:])

    nc.compile()
    _cache[key] = nc
    return nc


def _blkdiag(a):
    o = np.zeros((a.shape[0] * a.shape[1], a.shape[0]), np.float32)
    for h in range(a.shape[0]):
        o[h * a.shape[1]:(h + 1) * a.shape[1], h] = a[h]
    return o


def _core_lpt(deg, count_cap, pinned=None):
    """Assign nodes (degree desc) to NCORES cores, balancing total degree,
    count per core <= count_cap.  pinned[n] >= 0 forces node n's core.
    Returns core_of [n]."""
    import heapq
    core_of = np.zeros(len(deg), np.int64)
    loads = np.zeros(NCORES, np.int64)
    counts = np.zeros(NCORES, np.int64)
    free = np.ones(len(deg), bool)
    if pinned is not None:
        sel = pinned >= 0
        core_of[sel] = pinned[sel]
        free[sel] = False
        for r in range(NCORES):
            loads[r] = int(deg[(pinned == r)].sum())
            counts[r] = int((pinned == r).sum())
    order = np.argsort(-deg, kind='stable')
    heap = [(int(loads[r]), int(counts[r]), r) for r in range(NCORES)]
    heapq.heapify(heap)
    for d in order:
        if not free[d]:
            continue
        while True:
            load, cnt, r = heapq.heappop(heap)
            if cnt != counts[r]:
                continue  # stale entry
            if counts[r] < count_cap:
                break
        core_of[d] = r
        counts[r] += 1
        loads[r] += int(deg[d])
        heapq.heappush(heap, (int(loads[r]), int(counts[r]), r))
    return core_of


def _lpt_pack(dst_local, n_local, nwin, caps=None):
    """Pack dsts (by degree desc) into nwin windows (<=128 dsts each),
    respecting per-window edge-load caps when feasible.  Returns
    slot_of_dst [n_local] (w*128+p) and per-window edge loads."""
    deg = np.bincount(dst_local, minlength=n_local)
    order = np.argsort(-deg, kind='stable')
    loads = np.zeros(nwin, np.int64)
    counts = np.zeros(nwin, np.int64)
    slot = np.zeros(n_local, np.int64)
    import heapq
    heap = [(0, 0, w) for w in range(nwin)]
    heapq.heapify(heap)
    for d in order:
        dg = int(deg[d])
        popped = []
        chosen = None
        while heap:
            load, cnt, w = heapq.heappop(heap)
            if counts[w] >= P:
                continue  # full window: drop from heap for good
            if caps is not None and loads[w] + dg > caps[w]:
                popped.append((load, cnt, w))
                continue
            chosen = w
            break
        if chosen is None:
            # no window satisfies the cap: use min-load non-full window
            popped.sort()
            chosen = popped.pop(0)[2]
        for item in popped:
            heapq.heappush(heap, item)
        w = chosen
        slot[d] = w * P + counts[w]
        counts[w] += 1
        loads[w] += dg
        heapq.heappush(heap, (int(loads[w]), int(counts[w]), w))
    return slot, loads


def kernel(x, W1, al1, ar1, b1, W2, al2, ar2, b2, src0, dst0, src1, dst1):
    x = np.asarray(x, np.float32); W1 = np.asarray(W1, np.float32)
    al1 = np.asarray(al1, np.float32); ar1 = np.asarray(ar1, np.float32)
    b1 = np.asarray(b1, np.float32); W2 = np.asarray(W2, np.float32)
    al2 = np.asarray(al2, np.float32); ar2 = np.asarray(ar2, np.float32)
    b2 = np.asarray(b2, np.float32)
    src0 = np.asarray(src0, np.int32); dst0 = np.asarray(dst0, np.int32)
    src1 = np.asarray(src1, np.int32); dst1 = np.asarray(dst1, np.int32)

    W1Ev = np.concatenate([W1, W1 @ _blkdiag(al1)], axis=1).astype(BF)
    WARv = (W1 @ _blkdiag(ar1)).astype(BF)
    W2Ev = np.concatenate([W2, W2 @ _blkdiag(al2), W2 @ _blkdiag(ar2)],
                          axis=1).astype(BF)
    b1r = np.broadcast_to(b1.reshape(1, NF1), (P, NF1)).astype(np.float32).copy()
    b2m = b2.reshape(H, C).mean(axis=0)
    b2r = np.broadcast_to(b2m.reshape(1, C), (P, C)).astype(np.float32).copy()
    add_b1 = bool(np.any(b1)); add_b2 = bool(np.any(b2))
    xbf = x.astype(BF)

    # ---- node->core assignment: LPT by degree (count cap = slot count).
    # dst2 nodes are pinned to the same core for L1 and L2 so er2 comes
    # from the local G2in slab (no AllGather dependency).
    core2_of = _core_lpt(np.bincount(dst1, minlength=N2), SL2)
    deg1 = np.bincount(dst0, minlength=N1)
    pinned = np.full(N1, -1, np.int64)
    pinned[:N2] = core2_of
    core1_of = _core_lpt(deg1, SL1, pinned=pinned)
    core1 = core1_of[dst0]
    core2 = core2_of[dst1]

    # ---- per-core L1 window packing
    slot1 = np.zeros(N1, np.int64)       # dst1 -> w*128+p (within owning core)
    packs1 = []
    nodes1 = [np.flatnonzero(core1_of == r) for r in range(NCORES)]
    for r in range(NCORES):
        sel = core1 == r
        loc = np.full(N1, -1, np.int64)
        loc[nodes1[r]] = np.arange(len(nodes1[r]))
        dl = loc[dst0[sel]]
        s, loads = _lpt_pack(dl, len(nodes1[r]), NW1)
        slot1[nodes1[r]] = s
        packs1.append((np.flatnonzero(sel), dl, s, loads))
    # canonical window order: sort windows by load desc per core
    worder1 = [np.argsort(-p[3], kind='stable') for p in packs1]
    lw1 = np.stack([p[3][wo] for p, wo in zip(packs1, worder1)])  # [cores, NW1]
    tw1 = [max(1, int(np.ceil(lw1[:, w].max() / P))) for w in range(NW1)]
    # remap slot window ids to canonical order
    for r in range(NCORES):
        inv = np.empty(NW1, np.int64)
        inv[worder1[r]] = np.arange(NW1)
        s = slot1[nodes1[r]]
        slot1[nodes1[r]] = inv[s // P] * P + s % P

    # dst1 node -> G2 row under chunked-AllGather layout:
    # row = chunk*(8*CH*128) + core*(CH*128) + (w % CH)*128 + p
    _w = slot1 // P
    _p = slot1 % P
    _core = core1_of
    g2row = ((_w // CH) * (NCORES * CH * P) + _core * (CH * P)
             + (_w % CH) * P + _p)

    # ---- per-core L2 window packing
    slot2 = np.zeros(N2, np.int64)
    packs2 = []
    nodes2 = [np.flatnonzero(core2_of == r) for r in range(NCORES)]
    for r in range(NCORES):
        sel = core2 == r
        loc = np.full(N2, -1, np.int64)
        loc[nodes2[r]] = np.arange(len(nodes2[r]))
        dl = loc[dst1[sel]]
        s, loads = _lpt_pack(dl, len(nodes2[r]), NW2)
        slot2[nodes2[r]] = s
        packs2.append((np.flatnonzero(sel), dl, s, loads))
    worder2 = [np.argsort(-p[3], kind='stable') for p in packs2]
    lw2 = np.stack([p[3][wo] for p, wo in zip(packs2, worder2)])
    tw2 = [max(1, int(np.ceil(lw2[:, w].max() / P))) for w in range(NW2)]
    for r in range(NCORES):
        inv = np.empty(NW2, np.int64)
        inv[worder2[r]] = np.arange(NW2)
        s = slot2[nodes2[r]]
        slot2[nodes2[r]] = inv[s // P] * P + s % P

    # ---- chunk-pure L2 tiling: per (window, chunk) tile runs, profile
    # shared across cores (max) so the SPMD program lines up.
    CHROWS = NCORES * CH * P
    cnt2 = np.zeros((NCORES, NW2, NCH), np.int64)
    edata = []
    for r in range(NCORES):
        eidx2 = packs2[r][0]
        sl = slot2[dst1[eidx2]]
        wv = sl // P
        rows = g2row[src1[eidx2]]
        cv = rows // CHROWS
        np.add.at(cnt2[r], (wv, cv), 1)
        edata.append((sl, rows, cv, wv))
    ntile2 = np.ceil(cnt2.max(axis=0) / P).astype(np.int64)  # [NW2, NCH]
    for w in range(NW2):
        if ntile2[w].sum() == 0:
            ntile2[w, 0] = 1
    tw2 = [int(ntile2[w].sum()) for w in range(NW2)]
    chdep2 = []
    for w in range(NW2):
        for c in range(NCH):
            chdep2.extend([c] * int(ntile2[w, c]))
    T1, T2 = sum(tw1), sum(tw2)
    woff1 = np.concatenate([[0], np.cumsum(tw1)])
    woff2 = np.concatenate([[0], np.cumsum(tw2)])
    roff2 = np.zeros((NW2, NCH), np.int64)   # tile offset of run (w, c)
    for w in range(NW2):
        roff2[w] = woff2[w] + np.concatenate([[0], np.cumsum(ntile2[w])[:-1]])

    def pack_l2(r):
        sl, rows, cv, wv = edata[r]
        src_sl = np.zeros(T2 * P, np.int64)
        rd = np.full(T2 * P, PADRD, np.float32)
        order = np.lexsort((cv, wv))
        key = wv[order] * NCH + cv[order]
        bounds = np.searchsorted(key, np.arange(NW2 * NCH + 1))
        for w in range(NW2):
            for c in range(NCH):
                a, b = bounds[w * NCH + c], bounds[w * NCH + c + 1]
                n = b - a
                if n == 0:
                    continue
                idx = order[a:b]
                base = roff2[w, c] * P
                assert n <= ntile2[w, c] * P
                src_sl[base:base + n] = rows[idx] % CHROWS
                rd[base:base + n] = sl[idx] % P
        return src_sl, rd

    def pack_edges(eidx, srcrows, slots, nwin, tws, woff, T, sortkey=None):
        """Lay edges into window slot arrays.  Returns src_slot [T*P],
        rd [T*P] f32, tile_maxkey [T] (max sortkey per tile, 0 for empty)."""
        src_sl = np.zeros(T * P, np.int64)
        rd = np.full(T * P, PADRD, np.float32)
        tile_maxkey = np.zeros(T, np.int64)
        w_of_e = slots[:] // P
        if sortkey is None:
            order = np.argsort(w_of_e, kind='stable')
        else:
            order = np.lexsort((sortkey, w_of_e))
        bounds = np.searchsorted(w_of_e[order], np.arange(nwin + 1))
        for w in range(nwin):
            a, b = bounds[w], bounds[w + 1]
            idx = order[a:b]
            n = b - a
            base = woff[w] * P
            assert n <= tws[w] * P, (w, n, tws[w] * P)
            src_sl[base:base + n] = srcrows[idx]
            rd[base:base + n] = slots[idx] % P
            if sortkey is not None and n > 0:
                keys = sortkey[idx]
                for j in range((n + P - 1) // P):
                    tile_maxkey[woff[w] + j] = keys[min((j + 1) * P, n) - 1]
        return src_sl, rd, tile_maxkey

    in_maps = []
    for r in range(NCORES):
        eidx1, dl1, _, _ = packs1[r]
        sl_e1 = slot1[dst0[eidx1]]          # slot of each edge's dst
        src_sl1, rd1, _ = pack_edges(eidx1, src0[eidx1], sl_e1, NW1, tw1,
                                     woff1, T1)
        XE1v = xbf[src_sl1].T.copy()        # [256, T1*128]
        own1 = np.zeros(SL1, np.int64)      # window slot -> x row (own dsts)
        dsts_r = nodes1[r]
        own1[slot1[dsts_r]] = dsts_r
        XD1v = xbf[own1].T.copy()
        RD1v = rd1.astype(BF).reshape(1, -1).copy()
        MRC1v = rd1.astype(BF).reshape(T1, P).T.copy()

        eidx2 = packs2[r][0]
        src_sl2, rd2 = pack_l2(r)
        GIDX2v = src_sl2.astype(np.int32).reshape(T2, P).T.copy()
        RD2v = rd2.astype(BF).reshape(1, -1).copy()
        MRC2v = rd2.astype(BF).reshape(T2, P).T.copy()
        own2 = np.zeros(SL2, np.int64)
        dsts2_r = nodes2[r]
        own2[slot2[dsts2_r]] = slot1[dsts2_r]   # local G2in row
        E2Iv = own2.astype(np.int32).reshape(NW2, P).T.copy()

        in_maps.append(dict(
            XE1=XE1v, XD1=XD1v, RD1=RD1v, RD2=RD2v, MRC1=MRC1v, MRC2=MRC2v,
            GIDX2=GIDX2v, E2I=E2Iv, W1E=W1Ev, WAR=WARv, W2E=W2Ev,
            B1R=b1r, B2R=b2r))

    global _last_in_maps
    _last_in_maps = in_maps
    nc = build_program(tw1, tw2, add_b1, add_b2, chdep2)
    from concourse.bass_utils import run_bass_kernel_spmd
    res = None
    last_err = None
    for attempt in range(3):
        try:
            res = run_bass_kernel_spmd(nc, in_maps, core_ids=list(range(NCORES)))
            break
        except Exception as e:
            last_err = e
            import time as _t
            _t.sleep(10)
    if res is None:
        raise last_err
    out = np.zeros((N2, C), np.float32)
    for r in range(NCORES):
        o = res.results[r]["OUT"]
        dsts2_r = nodes2[r]
        out[dsts2_r] = o[slot2[dsts2_r]]
    return out
